# revision 17
# baseline (speedup 1.0000x reference)
"""DFSMN forward on 8 Trainium2 NeuronCores (Bass/Tile).

Math: the reference computes
    base_t = (1+l0)*v_t + sum_{k=1..10} r_{k-1} v_{t+k}
    p_t    = base_t + sum_{k=1..19} l_k p_{t-k}        (per-feature IIR)
which equals a per-feature convolution  p = g * v  with
    g_m[d] = h_m (1+l0) + sum_{k=1..10} h_{m+k} r_{k-1},  m in [-10, inf)
where h is the impulse response of the recurrence.  h decays geometrically,
so g is truncated: output column c (in-block time) covers lags up to
c + (Q-1)*128 - 10 (tapered window from the Q Toeplitz blocks).

Device mapping (per core, features sharded 8 ways -> 64 each):
  per feature d: a [128, BW] Toeplitz band (host-prebuilt, partition-major
  so DMA rows are multi-KB contiguous) is the stationary operand; the
  moving operand packs time%128 on partitions (order-flipped) and
  (batch x time-block) on the 512-wide free dim.  Q bf16 matmuls
  accumulate in fp32 PSUM, one small matmul applies the start-boundary
  correction, DVE/ACT convert PSUM to bf16, and 1-2MB DMAs stream
  everything; loads ride the sync HWDGE ring, stores the scalar ring.
  All wire data is bf16; the host packs/unpacks fp32<->bf16.
"""
import os
import numpy as np
import ml_dtypes

import concourse.bass as bass
import concourse.tile as tile
from concourse import bacc, mybir
from concourse.bass_utils import run_bass_kernel_spmd

BF = ml_dtypes.bfloat16

B, T, D = 32, 2048, 512
NCORES = 8
DLOC = D // NCORES          # 64 features per core
TB = T // 128               # 16 time blocks
BTB = B * TB                # 512 moving columns
KL, KR = 20, 10
Q = 2                       # Toeplitz blocks: lags covered up to c+118
BW = 128 * Q                # band width
MHI = BW - 10               # g table extent: m in [-10, MHI)
GPAD = BW + 128             # padded g-table row length
ECOLS = 256                 # host boundary-correction spans outputs t < 256
SH = KR                     # input packed time-shifted by +10 (causal kernel)
FG = 8                      # features per DMA group

LAST_EXEC_NS = None

_nc_cache = None


def _build_tables(l_filter: np.ndarray, r_filter: np.ndarray):
    l = l_filter.astype(np.float64)
    r = r_filter.astype(np.float64)
    H = MHI + 2 * KR
    h = np.zeros((H, D))
    h[0] = 1.0
    for j in range(1, H):
        kk = min(j, KL - 1)
        h[j] = np.einsum("kd,kd->d", l[1:kk + 1], h[j - kk:j][::-1])
    g = np.zeros((MHI + 10, D))
    for mi in range(MHI + 10):
        m = mi - 10
        acc = np.zeros(D)
        if m >= 0:
            acc = h[m] * (1.0 + l[0])
        for k in range(1, KR + 1):
            if m + k >= 0:
                acc = acc + h[m + k] * r[k - 1]
        g[mi] = acc
    gtab = np.zeros((D, GPAD), dtype=np.float64)
    gtab[:, 127:127 + MHI + 10] = g.T
    # combined boundary correction (outputs t < ECOLS), contracted with v[0:10]:
    #  + g[t-p]           : head restore (shifted packing drops v[0..9])
    #  - sum_{k>p} h[t-p+k] r[k-1] : FIR over-counts r-taps reading base t'<0
    ftab = np.zeros((D, KR, ECOLS), dtype=np.float64)
    tt = np.arange(ECOLS)
    for p in range(KR):
        acc = np.zeros((ECOLS, D))
        m = tt - p
        sel = (m >= -10) & (m < MHI)
        acc[sel] += g[m[sel] + 10]
        for k in range(p + 1, KR + 1):
            acc -= h[tt - p + k] * r[k - 1]
        ftab[:, p, :] = acc.T
    # ftab stays fp32: the boundary correction is applied on the host
    return gtab.astype(BF), ftab.astype(np.float32)


def _build_bass():
    nc = bacc.Bacc("TRN2", target_bir_lowering=False, debug=False)
    bf = mybir.dt.bfloat16
    xin = nc.dram_tensor("xin", [128, DLOC, BTB], bf, kind="ExternalInput")
    bd = nc.dram_tensor("bd", [128, DLOC, BW], bf, kind="ExternalInput")
    ot = nc.dram_tensor("ot", [128, DLOC, BTB], bf, kind="ExternalOutput")
    with tile.TileContext(nc) as tc:
        with tc.tile_pool(name="x", bufs=3) as xp, \
             tc.tile_pool(name="w", bufs=3) as wp, \
             tc.tile_pool(name="o", bufs=3) as op_, \
             tc.tile_pool(name="ps", bufs=8, space="PSUM") as pp:
            for gi, g0 in enumerate(range(0, DLOC, FG)):
                # x and band loads ride opposite HWDGE rings (alternating
                # per group); stores take the gpsimd SWDGE ring, so loads
                # never queue behind stores
                xeng = nc.sync if gi % 2 == 0 else nc.scalar
                beng = nc.scalar if gi % 2 == 0 else nc.sync
                bg = wp.tile([128, FG * BW], bf)
                beng.dma_start(
                    out=bg[:],
                    in_=bd[:, g0:g0 + FG, :].rearrange("p f w -> p (f w)"))
                xg = xp.tile([128, FG * BTB], bf)
                xeng.dma_start(
                    out=xg[:],
                    in_=xin[:, g0:g0 + FG, :].rearrange("p f w -> p (f w)"))
                og = op_.tile([128, FG * BTB], bf)
                for fi in range(FG):
                    ps = pp.tile([128, BTB], mybir.dt.float32)
                    xv = xg[:, fi * BTB:(fi + 1) * BTB]
                    x3 = xv.rearrange("p (b t) -> p b t", b=B)
                    p3 = ps[:].rearrange("p (b t) -> p b t", b=B)
                    # q=0: same-block (shifted packing makes kernel causal)
                    nc.tensor.matmul(ps[:], bg[:, fi * BW:fi * BW + 128], xv,
                                     start=True, stop=Q == 1)
                    for q in range(1, Q):
                        nc.tensor.matmul(
                            p3[:, :, q:TB],
                            bg[:, fi * BW + 128 * q:fi * BW + 128 * (q + 1)],
                            x3[:, :, 0:TB - q],
                            start=False, stop=q == Q - 1)
                    dst = og[:, fi * BTB:(fi + 1) * BTB]
                    if fi % 2 == 1:
                        nc.scalar.copy(dst, ps[:])
                    else:
                        nc.vector.tensor_copy(dst, ps[:])
                nc.gpsimd.dma_start(
                    out=ot[:, g0:g0 + FG, :].rearrange("p f w -> p (f w)"),
                    in_=og[:])
    nc.compile()
    return nc


def kernel(v: np.ndarray, l_filter: np.ndarray, r_filter: np.ndarray) -> np.ndarray:
    global _nc_cache, LAST_EXEC_NS
    v = np.asarray(v, dtype=np.float32)
    gtab, etab = _build_tables(np.asarray(l_filter), np.asarray(r_filter))

    # Toeplitz bands, partition-major: band[i, d, c] = gtab[d, i + c]
    band = np.lib.stride_tricks.sliding_window_view(gtab, BW, axis=1)
    band = np.ascontiguousarray(band[:, :128, :].transpose(1, 0, 2))

    # pack v: [B,1,T,D] -> [i, d, b*TB+tb] bf16, partition i = flipped
    # in-block time (t = tb*128 + 127 - i), shifted so x[t'] = v[t'+SH]
    s = v[:, 0, :, :]                                  # [B, T, D]
    ssh = np.zeros_like(s)
    ssh[:, :T - SH, :] = s[:, SH:, :]
    tiles = ssh.reshape(B, TB, 128, D)[:, :, ::-1, :]  # [b, tb, i, d]
    xall = np.ascontiguousarray(
        tiles.transpose(2, 3, 0, 1)).reshape(128, D, BTB).astype(BF)

    if _nc_cache is None:
        _nc_cache = _build_bass()
    nc = _nc_cache

    in_maps = []
    for c in range(NCORES):
        lo, hi = c * DLOC, (c + 1) * DLOC
        in_maps.append({
            "xin": np.ascontiguousarray(xall[:, lo:hi, :]),
            "bd": np.ascontiguousarray(band[:, lo:hi, :]),
        })
    trace = os.environ.get("DFSMN_TRACE", "0") == "1"
    r = run_bass_kernel_spmd(nc, in_maps, list(range(NCORES)), trace=trace)
    LAST_EXEC_NS = r.exec_time_ns
    ot_all = np.concatenate(
        [np.asarray(r.results[c]["ot"]) for c in range(NCORES)], axis=1)

    out = ot_all.reshape(128, D, B, TB).transpose(2, 3, 0, 1) \
        .reshape(B, T, D).astype(np.float32)
    # host-side start-boundary correction (fp32), t < ECOLS:
    # out[b,t,d] += sum_p ftab[d,p,t] * v[b,p,d]
    out[:, :ECOLS, :] += np.einsum(
        "dpt,bpd->btd", etab, s[:, :KR, :], optimize=True)
    return np.ascontiguousarray(out[:, None, :, :])


# revision 20
# speedup vs baseline: 1.1381x; 1.1381x over previous
"""DFSMN forward on 8 Trainium2 NeuronCores (Bass/Tile).

Math: the reference computes
    base_t = (1+l0)*v_t + sum_{k=1..10} r_{k-1} v_{t+k}
    p_t    = base_t + sum_{k=1..19} l_k p_{t-k}        (per-feature IIR)
which equals a per-feature convolution  p = g * v  with
    g_m[d] = h_m (1+l0) + sum_{k=1..10} h_{m+k} r_{k-1},  m in [-10, inf)
where h is the impulse response of the recurrence.  h decays geometrically,
so g is truncated: output column c (in-block time) covers lags up to
c + (Q-1)*128 - 10 (tapered window from the Q Toeplitz blocks).

Device mapping (per core, features sharded 8 ways -> 64 each):
  per feature d: a [128, BW] Toeplitz band (host-prebuilt, partition-major
  so DMA rows are multi-KB contiguous) is the stationary operand; the
  moving operand packs time%128 on partitions (order-flipped) and
  (batch x time-block) on the 512-wide free dim.  Q bf16 matmuls
  accumulate in fp32 PSUM, one small matmul applies the start-boundary
  correction, DVE/ACT convert PSUM to bf16, and 1-2MB DMAs stream
  everything; loads ride the sync HWDGE ring, stores the scalar ring.
  All wire data is bf16; the host packs/unpacks fp32<->bf16.
"""
import os
import numpy as np
import ml_dtypes

import concourse.bass as bass
import concourse.tile as tile
from concourse import bacc, mybir
from concourse.bass_utils import run_bass_kernel_spmd

BF = ml_dtypes.bfloat16

B, T, D = 32, 2048, 512
NCORES = 8
DLOC = D // NCORES          # 64 features per core
TB = T // 128               # 16 time blocks
BTB = B * TB                # 512 moving columns
KL, KR = 20, 10
Q = 2                       # Toeplitz blocks: lags covered up to c+118
BW = 128 * Q                # band width
MHI = BW - 10               # g table extent: m in [-10, MHI)
GPAD = BW + 128             # padded g-table row length
ECOLS = 256                 # host boundary-correction spans outputs t < 256
SH = KR                     # input packed time-shifted by +10 (causal kernel)
# feature-group sizes: small tail groups whose stores fan out across three
# DMA rings drain the pipeline concurrently instead of one 1MB store
FGS = [8] * 7 + [4, 2, 2]
assert sum(FGS) == DLOC

LAST_EXEC_NS = None

_nc_cache = None


def _build_tables(l_filter: np.ndarray, r_filter: np.ndarray):
    l = l_filter.astype(np.float64)
    r = r_filter.astype(np.float64)
    H = MHI + 2 * KR
    h = np.zeros((H, D))
    h[0] = 1.0
    for j in range(1, H):
        kk = min(j, KL - 1)
        h[j] = np.einsum("kd,kd->d", l[1:kk + 1], h[j - kk:j][::-1])
    g = np.zeros((MHI + 10, D))
    for mi in range(MHI + 10):
        m = mi - 10
        acc = np.zeros(D)
        if m >= 0:
            acc = h[m] * (1.0 + l[0])
        for k in range(1, KR + 1):
            if m + k >= 0:
                acc = acc + h[m + k] * r[k - 1]
        g[mi] = acc
    gtab = np.zeros((D, GPAD), dtype=np.float64)
    gtab[:, 127:127 + MHI + 10] = g.T
    # combined boundary correction (outputs t < ECOLS), contracted with v[0:10]:
    #  + g[t-p]           : head restore (shifted packing drops v[0..9])
    #  - sum_{k>p} h[t-p+k] r[k-1] : FIR over-counts r-taps reading base t'<0
    ftab = np.zeros((D, KR, ECOLS), dtype=np.float64)
    tt = np.arange(ECOLS)
    for p in range(KR):
        acc = np.zeros((ECOLS, D))
        m = tt - p
        sel = (m >= -10) & (m < MHI)
        acc[sel] += g[m[sel] + 10]
        for k in range(p + 1, KR + 1):
            acc -= h[tt - p + k] * r[k - 1]
        ftab[:, p, :] = acc.T
    # ftab stays fp32: the boundary correction is applied on the host
    return gtab.astype(BF), ftab.astype(np.float32)


def _build_bass():
    nc = bacc.Bacc("TRN2", target_bir_lowering=False, debug=False)
    bf = mybir.dt.bfloat16
    xin = nc.dram_tensor("xin", [128, DLOC, BTB], bf, kind="ExternalInput")
    bd = nc.dram_tensor("bd", [128, DLOC, BW], bf, kind="ExternalInput")
    ot = nc.dram_tensor("ot", [128, DLOC, BTB], bf, kind="ExternalOutput")
    with tile.TileContext(nc) as tc:
        with tc.tile_pool(name="x", bufs=3) as xp, \
             tc.tile_pool(name="w", bufs=3) as wp, \
             tc.tile_pool(name="o", bufs=3) as op_, \
             tc.tile_pool(name="ps", bufs=8, space="PSUM") as pp:
            g0 = 0
            for gi, fg in enumerate(FGS):
                # x and band loads ride opposite HWDGE rings (alternating
                # per group); stores take the gpsimd SWDGE ring, so loads
                # never queue behind stores (tail stores fan out, below)
                xeng = nc.sync if gi % 2 == 0 else nc.scalar
                beng = nc.scalar if gi % 2 == 0 else nc.sync
                bg = wp.tile([128, fg * BW], bf, tag="bg")
                beng.dma_start(
                    out=bg[:],
                    in_=bd[:, g0:g0 + fg, :].rearrange("p f w -> p (f w)"))
                xg = xp.tile([128, fg * BTB], bf, tag="xg")
                xeng.dma_start(
                    out=xg[:],
                    in_=xin[:, g0:g0 + fg, :].rearrange("p f w -> p (f w)"))
                og = op_.tile([128, fg * BTB], bf, tag="og")
                for fi in range(fg):
                    ps = pp.tile([128, BTB], mybir.dt.float32)
                    xv = xg[:, fi * BTB:(fi + 1) * BTB]
                    x3 = xv.rearrange("p (b t) -> p b t", b=B)
                    p3 = ps[:].rearrange("p (b t) -> p b t", b=B)
                    # q=0: same-block (shifted packing makes kernel causal)
                    nc.tensor.matmul(ps[:], bg[:, fi * BW:fi * BW + 128], xv,
                                     start=True, stop=Q == 1)
                    for q in range(1, Q):
                        nc.tensor.matmul(
                            p3[:, :, q:TB],
                            bg[:, fi * BW + 128 * q:fi * BW + 128 * (q + 1)],
                            x3[:, :, 0:TB - q],
                            start=False, stop=q == Q - 1)
                    dst = og[:, fi * BTB:(fi + 1) * BTB]
                    if fi % 2 == 1:
                        nc.scalar.copy(dst, ps[:])
                    else:
                        nc.vector.tensor_copy(dst, ps[:])
                seng = nc.gpsimd if gi < len(FGS) - 2 else \
                    (nc.scalar if gi == len(FGS) - 2 else nc.sync)
                seng.dma_start(
                    out=ot[:, g0:g0 + fg, :].rearrange("p f w -> p (f w)"),
                    in_=og[:])
                g0 += fg
    nc.compile()
    return nc


def kernel(v: np.ndarray, l_filter: np.ndarray, r_filter: np.ndarray) -> np.ndarray:
    global _nc_cache, LAST_EXEC_NS
    v = np.asarray(v, dtype=np.float32)
    gtab, etab = _build_tables(np.asarray(l_filter), np.asarray(r_filter))

    # Toeplitz bands, partition-major: band[i, d, c] = gtab[d, i + c]
    band = np.lib.stride_tricks.sliding_window_view(gtab, BW, axis=1)
    band = np.ascontiguousarray(band[:, :128, :].transpose(1, 0, 2))

    # pack v: [B,1,T,D] -> [i, d, b*TB+tb] bf16, partition i = flipped
    # in-block time (t = tb*128 + 127 - i), shifted so x[t'] = v[t'+SH]
    s = v[:, 0, :, :]                                  # [B, T, D]
    ssh = np.zeros_like(s)
    ssh[:, :T - SH, :] = s[:, SH:, :]
    tiles = ssh.reshape(B, TB, 128, D)[:, :, ::-1, :]  # [b, tb, i, d]
    xall = np.ascontiguousarray(
        tiles.transpose(2, 3, 0, 1)).reshape(128, D, BTB).astype(BF)

    if _nc_cache is None:
        _nc_cache = _build_bass()
    nc = _nc_cache

    in_maps = []
    for c in range(NCORES):
        lo, hi = c * DLOC, (c + 1) * DLOC
        in_maps.append({
            "xin": np.ascontiguousarray(xall[:, lo:hi, :]),
            "bd": np.ascontiguousarray(band[:, lo:hi, :]),
        })
    trace = os.environ.get("DFSMN_TRACE", "0") == "1"
    r = run_bass_kernel_spmd(nc, in_maps, list(range(NCORES)), trace=trace)
    LAST_EXEC_NS = r.exec_time_ns
    ot_all = np.concatenate(
        [np.asarray(r.results[c]["ot"]) for c in range(NCORES)], axis=1)

    out = ot_all.reshape(128, D, B, TB).transpose(2, 3, 0, 1) \
        .reshape(B, T, D).astype(np.float32)
    # host-side start-boundary correction (fp32), t < ECOLS:
    # out[b,t,d] += sum_p ftab[d,p,t] * v[b,p,d]
    out[:, :ECOLS, :] += np.einsum(
        "dpt,bpd->btd", etab, s[:, :KR, :], optimize=True)
    return np.ascontiguousarray(out[:, None, :, :])
